# revision 14
# baseline (speedup 1.0000x reference)
"""SATD loss kernel for Trainium2: sum |H @ (original - pred)|.

Full inputs: original, pred [2, 8192, 64, 64] f32. H is the 64x64
Sylvester Hadamard matrix applied along axis -2 of each 64x64 block.

Strategy (8-way data parallel over the 16384 blocks, 2048 per core):
  - Host: shard blocks across cores, cast original and NEGATED pred to
    fp8 e4m3, repack each core's data into [T, 128, COLS] tiles whose
    partition axis holds the j-rows of 128 blocks (two 64-block halves
    m=0/1 on partitions 0-63 / 64-127) and whose free axis is (g, k).
  - Device, per tile: load `a` with a 1 MiB HWDGE DMA on the SP ring,
    then accumulate `-b` onto the same SBUF tile with a SWDGE
    (gpsimd-ring) DMA using accum_op=add -- the inline CCE ALU computes
    diff = a - b in fp32 and requantizes to fp8 during the transfer.
    This halves the TensorE streaming work vs feeding a and b
    separately: per 512-column group one plain fp8 matmul with
    lhsT = kron(I2, H) computes H @ diff (~0.25us instead of ~0.38us
    DoubleRow on 2x the stream), leaving PE far from critical.
  - Two groups share a 2-bank [128, 1024] PSUM tile (4 such tiles
    cycle through the 8 PSUM banks so matmul fill and reduce overlap);
    one fused abs+sum per tile, split 17:15 between VectorE
    tensor_reduce (apply_absolute_value, ~1.21us) and ScalarE
    activation(Abs, accum_out, ~1.40us) so both engines finish
    together, just under the HBM streaming time (~40us for 16.8 MB).
  - Final reduce -> [128, 2]/core; host sums the 8x128 partials (f64).
"""

import os
from contextlib import ExitStack

import ml_dtypes
import numpy as np

import concourse.bass as bass
import concourse.tile as tile
from concourse import bacc, mybir
from concourse.bass_utils import run_bass_kernel_spmd

N_CORES = 8
N = 64                       # Hadamard block size
BLOCKS_TOTAL = 2 * 8192      # 16384 blocks of [64, 64]
BLOCKS_PER_CORE = BLOCKS_TOTAL // N_CORES   # 2048
G = 256                      # blocks per tile (two 128-block halves)
COLS = (G // 2) * N          # 8192 fp8 per input per partition per tile
TILES = BLOCKS_PER_CORE // G                # 8 tiles of 1 MiB
MM_N = 512                   # matmul moving free dim (one PSUM bank)
SUB = COLS // MM_N           # matmul groups per tile (16)
QUAD = 2                     # groups per reduce op (2 PSUM banks)
NQ = TILES * SUB // QUAD     # reduce ops total (64)

F32 = mybir.dt.float32
IN_DT = mybir.dt.float8e4
IN_NP = ml_dtypes.float8_e4m3

# Tunables (env overrides are for local A/B experiments only; the
# defaults are what the kernel ships with).
XBUFS = int(os.environ.get("SATD_XBUFS", "4"))
PSUM_BUFS = int(os.environ.get("SATD_PSUM_BUFS", "4"))
# Of every 32 reduce ops, this many go to VectorE (rest to ScalarE).
# Measured per-op cost: vector 1214 ns vs scalar 1108+288 = 1396 ns at
# FD=1024, so vector takes 1396/(1214+1396) ~= 53.5% of the ops.
VEC_OF_32 = int(os.environ.get("SATD_VEC32", "17"))


def _hadamard(n: int) -> np.ndarray:
    H = np.array([[1.0]], dtype=np.float32)
    while H.shape[0] < n:
        H = np.block([[H, H], [H, -H]])
    return H.astype(np.float32)


def _weights() -> np.ndarray:
    # lhsT for out = Hd @ rhs is Hd.T; kron(I2, H) is symmetric.
    Hd = np.kron(np.eye(2, dtype=np.float32), _hadamard(N))
    return Hd.astype(IN_NP)  # [128, 128], entries +-1 exact in fp8


def _vec_pattern():
    """Spread VEC_OF_32 vector-ops evenly through every 32 reduce ops."""
    pat = []
    acc = 0
    for k in range(32):
        nxt = (k + 1) * VEC_OF_32 // 32
        pat.append(nxt > acc)
        acc = nxt
    return pat


def _build_program() -> bacc.Bacc:
    nc = bacc.Bacc("TRN2", target_bir_lowering=False, debug=False,
                   num_devices=N_CORES)
    xa = nc.dram_tensor("xa", [TILES, 128, COLS], IN_DT,
                        kind="ExternalInput").ap()
    xnb = nc.dram_tensor("xnb", [TILES, 128, COLS], IN_DT,
                         kind="ExternalInput").ap()
    w = nc.dram_tensor("w", [128, 128], IN_DT, kind="ExternalInput").ap()
    out = nc.dram_tensor("out", [128, 2], F32, kind="ExternalOutput").ap()

    pat = _vec_pattern()
    nv_total = sum(1 for k in range(NQ) if pat[k % 32])
    na_total = NQ - nv_total

    with tile.TileContext(nc) as tc, ExitStack() as ctx:
        wpool = ctx.enter_context(tc.tile_pool(name="w", bufs=1))
        xpool = ctx.enter_context(tc.tile_pool(name="x", bufs=XBUFS))
        psum = ctx.enter_context(tc.tile_pool(name="psum", bufs=PSUM_BUFS,
                                              space="PSUM"))
        accpool = ctx.enter_context(tc.tile_pool(name="acc", bufs=1))
        scratch = ctx.enter_context(tc.tile_pool(name="scr", bufs=2))

        wt = wpool.tile([128, 128], IN_DT)
        nc.sync.dma_start(wt[:], w[:])

        accv = accpool.tile([128, max(nv_total, 1)], F32, tag="accv")
        acca = accpool.tile([128, max(na_total, 1)], F32, tag="acca")

        nv = 0
        na = 0
        for t in range(TILES):
            xt = xpool.tile([128, COLS], IN_DT)
            # a on the SP HWDGE ring, then -b accumulated onto the same
            # tile on the gpsimd SWDGE ring (inline CCE add => diff).
            # The CCE caps accumulate descriptors at 2048 elements per
            # partition, so the accum side always moves in 2048-column
            # chunks; the a side uses one DMA per tile (halves for the
            # first two tiles so the first matmuls start sooner).
            n_chunks = 2 if t < 2 else 1
            step = COLS // n_chunks
            for lo in range(0, COLS, step):
                nc.sync.dma_start(xt[:, lo:lo + step], xa[t, :, lo:lo + step])
            for lo in range(0, COLS, 2048):
                nc.gpsimd.dma_start(xt[:, lo:lo + 2048],
                                    xnb[t, :, lo:lo + 2048],
                                    accum_op=mybir.AluOpType.add)
            # Per 512-column group one plain fp8 matmul computes
            # Hd @ diff. Two groups share a 2-bank PSUM tile and one
            # abs+sum.
            for qd in range(SUB // QUAD):
                pt = psum.tile([128, QUAD * MM_N], F32)
                for j in range(QUAD):
                    s = qd * QUAD + j
                    nc.tensor.matmul(pt[:, j * MM_N:(j + 1) * MM_N], wt[:],
                                     xt[:, s * MM_N:(s + 1) * MM_N],
                                     start=True, stop=True)
                k = t * (SUB // QUAD) + qd
                if pat[k % 32]:
                    nc.vector.tensor_reduce(
                        accv[:, nv:nv + 1], pt[:],
                        axis=mybir.AxisListType.X, op=mybir.AluOpType.add,
                        apply_absolute_value=True)
                    nv += 1
                else:
                    st = scratch.tile([128, QUAD * MM_N], F32)
                    nc.scalar.activation(
                        st[:], pt[:], mybir.ActivationFunctionType.Abs,
                        accum_out=acca[:, na:na + 1])
                    na += 1

        res = accpool.tile([128, 2], F32, tag="res")
        nc.vector.tensor_reduce(res[:, 0:1], accv[:],
                                axis=mybir.AxisListType.X,
                                op=mybir.AluOpType.add)
        nc.vector.tensor_reduce(res[:, 1:2], acca[:],
                                axis=mybir.AxisListType.X,
                                op=mybir.AluOpType.add)
        nc.sync.dma_start(out[:], res[:])

    nc.compile()
    return nc


def _repack(shard: np.ndarray) -> np.ndarray:
    """[BLOCKS_PER_CORE, 64, 64] -> [TILES, 128, COLS] with partition
    axis (m, j) over two 64-block halves and free axis (g, k)."""
    v = shard.reshape(TILES, 2, G // 2, N, N)     # t, m, g, j, k
    v = v.transpose(0, 1, 3, 2, 4)                # t, m, j, g, k
    return v.reshape(TILES, 128, COLS)


_NC = None


def _get_program() -> bacc.Bacc:
    global _NC
    if _NC is None:
        _NC = _build_program()
    return _NC


def _run(original: np.ndarray, pred: np.ndarray, **spmd_kwargs):
    a_full = np.asarray(original, dtype=np.float32).reshape(
        BLOCKS_TOTAL, N, N).astype(IN_NP)
    nb_full = (-np.asarray(pred, dtype=np.float32).reshape(
        BLOCKS_TOTAL, N, N)).astype(IN_NP)
    wnp = _weights()
    in_maps = []
    for i in range(N_CORES):
        sl = slice(i * BLOCKS_PER_CORE, (i + 1) * BLOCKS_PER_CORE)
        in_maps.append({"xa": _repack(a_full[sl]),
                        "xnb": _repack(nb_full[sl]), "w": wnp})
    nc = _get_program()
    r = run_bass_kernel_spmd(nc, in_maps, list(range(N_CORES)),
                             **spmd_kwargs)
    total = 0.0
    for i in range(N_CORES):
        total += r.results[i]["out"].astype(np.float64).sum()
    return np.float32(total), r


def kernel(original: np.ndarray, pred: np.ndarray) -> np.ndarray:
    val, _ = _run(original, pred)
    return np.array(val, dtype=np.float32)


# revision 21
# speedup vs baseline: 1.7130x; 1.7130x over previous
"""SATD loss kernel for Trainium2: sum |H @ (original - pred)|.

Full inputs: original, pred [2, 8192, 64, 64] f32. H is the 64x64
Sylvester Hadamard matrix applied along axis -2 of each 64x64 block.

Strategy (8-way data parallel over the 16384 blocks, 2048 per core):
  - Host: shard blocks across cores, cast to fp8 e4m3 (the transform
    accumulates in fp32 PSUM; quantization contributes ~4e-4 relative
    error on the final scalar), and repack each core's data into
    [T, 128, 2*COLS] tiles whose partition axis holds the j-rows of 256
    blocks (two 128-block halves m=0/1 on partitions 0-63 / 64-127) and
    whose free axis interleaves original/pred per 512-column group.
  - Device, per 2 MiB tile: one DMA on the SP HWDGE ring (it streams
    back-to-back at ~425 GB/s); per 512-column group, a single
    DoubleRow fp8 matmul with lhsT = [kron(I2,H) | -kron(I2,H)]
    computes H @ (A - B) into PSUM. The first x chunk is dispatched
    before the weight DMA and the first two tiles stream in small
    chunks so the PE pipeline starts as early as possible.
  - Two groups share a 2-bank [128, 1024] PSUM tile (4 such tiles
    cycle through the 8 PSUM banks so matmul fill and reduce overlap);
    one fused abs+sum per tile, split 17:15 between VectorE
    tensor_reduce (apply_absolute_value, ~1.21us) and ScalarE
    activation(Abs, accum_out, ~1.40us) so both engines finish
    together, just under the TensorE streaming time.
  - Final reduce -> [128, 2]/core; host sums the 8x128 partials (f64).
"""

import os
from contextlib import ExitStack

import ml_dtypes
import numpy as np

import concourse.bass as bass
import concourse.tile as tile
from concourse import bacc, mybir
from concourse.bass_utils import run_bass_kernel_spmd

N_CORES = 8
N = 64                       # Hadamard block size
BLOCKS_TOTAL = 2 * 8192      # 16384 blocks of [64, 64]
BLOCKS_PER_CORE = BLOCKS_TOTAL // N_CORES   # 2048
G = 128                      # blocks per partition-half per tile
COLS = G * N                 # 8192 fp8 per input per partition per tile
TILES = BLOCKS_PER_CORE // (2 * G)          # 8 tiles of 2 MiB
MM_N = 512                   # matmul moving free dim (one PSUM bank)
SUB = COLS // MM_N           # matmul groups per tile (16)
QUAD = 2                     # groups per reduce op (2 PSUM banks)
NQ = TILES * SUB // QUAD     # reduce ops total (64)

F32 = mybir.dt.float32
IN_DT = mybir.dt.float8e4
IN_NP = ml_dtypes.float8_e4m3

# Tunables (env overrides are for local A/B experiments only; the
# defaults are what the kernel ships with).
XBUFS = int(os.environ.get("SATD_XBUFS", "3"))
PSUM_BUFS = int(os.environ.get("SATD_PSUM_BUFS", "4"))
# Of every 32 reduce ops, this many go to VectorE (rest to ScalarE).
# Measured per-op cost: vector 1214 ns vs scalar 1108+288 = 1396 ns at
# FD=1024, so vector takes 1396/(1214+1396) ~= 53.5% of the ops.
VEC_OF_32 = int(os.environ.get("SATD_VEC32", "17"))
# Number of warm-up matmuls on junk weights issued while the first
# data DMA is in flight, to lift the PE HAM clock gate to 8/8 before
# real matmuls start (cold MMs run at half clock).
WARMUP_MMS = int(os.environ.get("SATD_WARMUP", "10"))


def _hadamard(n: int) -> np.ndarray:
    H = np.array([[1.0]], dtype=np.float32)
    while H.shape[0] < n:
        H = np.block([[H, H], [H, -H]])
    return H.astype(np.float32)


def _weights() -> np.ndarray:
    # lhsT for out = Hd @ rhs is Hd.T; kron(I2, H) is symmetric.
    Hd = np.kron(np.eye(2, dtype=np.float32), _hadamard(N))
    return np.concatenate([Hd, -Hd], axis=1).astype(
        IN_NP)  # [128, 256], entries +-1 exact in fp8


def _vec_pattern():
    """Spread VEC_OF_32 vector-ops evenly through every 32 reduce ops."""
    pat = []
    acc = 0
    for k in range(32):
        nxt = (k + 1) * VEC_OF_32 // 32
        pat.append(nxt > acc)
        acc = nxt
    return pat


def _build_program() -> bacc.Bacc:
    nc = bacc.Bacc("TRN2", target_bir_lowering=False, debug=False,
                   num_devices=N_CORES)
    x = nc.dram_tensor("x", [TILES, 128, 2 * COLS], IN_DT,
                       kind="ExternalInput").ap()
    w = nc.dram_tensor("w", [128, 256], IN_DT, kind="ExternalInput").ap()
    out = nc.dram_tensor("out", [128, 2], F32, kind="ExternalOutput").ap()

    pat = _vec_pattern()
    nv_total = sum(1 for k in range(NQ) if pat[k % 32])
    na_total = NQ - nv_total

    with tile.TileContext(nc) as tc, ExitStack() as ctx:
        wpool = ctx.enter_context(tc.tile_pool(name="w", bufs=1))
        xpool = ctx.enter_context(tc.tile_pool(name="x", bufs=XBUFS))
        psum = ctx.enter_context(tc.tile_pool(name="psum", bufs=PSUM_BUFS,
                                              space="PSUM"))
        accpool = ctx.enter_context(tc.tile_pool(name="acc", bufs=1))
        scratch = ctx.enter_context(tc.tile_pool(name="scr", bufs=2))

        # First 256 KiB data chunk ahead of everything (it gates the
        # first matmul), then the small weight DMA, then the rest.
        xt0 = xpool.tile([128, 2 * COLS], IN_DT, tag="x0")
        nc.sync.dma_start(xt0[:, 0:2048], x[0, :, 0:2048])
        wt = wpool.tile([128, 256], IN_DT)
        nc.sync.dma_start(wt[:], w[:])

        accv = accpool.tile([128, max(nv_total, 1)], F32, tag="accv")
        acca = accpool.tile([128, max(na_total, 1)], F32, tag="acca")

        w3 = wt[:].rearrange("p (h m) -> p h m", h=2)

        nv = 0
        na = 0
        for t in range(TILES):
            xt = xt0 if t == 0 else xpool.tile([128, 2 * COLS], IN_DT)
            # Host layout interleaves the original/pred halves per
            # 512-column group: xt cols = (s, h, c). Any contiguous
            # column range is then self-contained. The first two tiles
            # stream in chunks so the pipeline fills quickly; the rest
            # use one 2 MiB DMA each (the SP ring streams them
            # back-to-back at ~425 GB/s).
            if t == 0:
                bounds = [2048, 4096, 8192, 16384]  # first chunk issued above
            elif t == 1:
                bounds = [8192, 16384]
            else:
                bounds = [16384]
            lo = 0 if t != 0 else 2048
            for hi in bounds:
                if hi > lo:
                    nc.sync.dma_start(xt[:, lo:hi], x[t, :, lo:hi])
                lo = hi
            # DoubleRow contracts over (p, h) in one pass: a single
            # matmul computes Hd@A - Hd@B per 512-column group. Two
            # groups share a 2-bank PSUM tile and one abs+sum.
            for qd in range(SUB // QUAD):
                pt = psum.tile([128, QUAD * MM_N], F32)
                for j in range(QUAD):
                    s = qd * QUAD + j
                    x3 = xt[:, s * 2 * MM_N:(s + 1) * 2 * MM_N].rearrange(
                        "p (h c) -> p h c", h=2)
                    nc.tensor.matmul(pt[:, j * MM_N:(j + 1) * MM_N], w3, x3,
                                     start=True, stop=True,
                                     perf_mode=mybir.MatmulPerfMode.DoubleRow)
                k = t * (SUB // QUAD) + qd
                if pat[k % 32]:
                    nc.vector.tensor_reduce(
                        accv[:, nv:nv + 1], pt[:],
                        axis=mybir.AxisListType.X, op=mybir.AluOpType.add,
                        apply_absolute_value=True)
                    nv += 1
                else:
                    st = scratch.tile([128, QUAD * MM_N], F32)
                    nc.scalar.activation(
                        st[:], pt[:], mybir.ActivationFunctionType.Abs,
                        accum_out=acca[:, na:na + 1])
                    na += 1

        res = accpool.tile([128, 2], F32, tag="res")
        nc.vector.tensor_reduce(res[:, 0:1], accv[:],
                                axis=mybir.AxisListType.X,
                                op=mybir.AluOpType.add)
        nc.vector.tensor_reduce(res[:, 1:2], acca[:],
                                axis=mybir.AxisListType.X,
                                op=mybir.AluOpType.add)
        nc.sync.dma_start(out[:], res[:])

    nc.compile()
    return nc


def _repack(shard: np.ndarray) -> np.ndarray:
    """[BLOCKS_PER_CORE, 64, 64] -> [TILES, 128, SUB, MM_N] with
    partition axis (m, j) and free axis (g, k) split into SUB groups of
    512 columns (8 g-blocks each)."""
    v = shard.reshape(TILES, 2, G, N, N)          # t, m, g, j, k
    v = v.transpose(0, 1, 3, 2, 4)                # t, m, j, g, k
    return v.reshape(TILES, 128, SUB, MM_N)


_NC = None


def _get_program() -> bacc.Bacc:
    global _NC
    if _NC is None:
        _NC = _build_program()
    return _NC


def _run(original: np.ndarray, pred: np.ndarray, **spmd_kwargs):
    a_full = np.asarray(original, dtype=np.float32).reshape(
        BLOCKS_TOTAL, N, N).astype(IN_NP)
    b_full = np.asarray(pred, dtype=np.float32).reshape(
        BLOCKS_TOTAL, N, N).astype(IN_NP)
    wnp = _weights()
    in_maps = []
    for i in range(N_CORES):
        sl = slice(i * BLOCKS_PER_CORE, (i + 1) * BLOCKS_PER_CORE)
        xi = np.empty((TILES, 128, SUB, 2, MM_N), dtype=IN_NP)
        xi[:, :, :, 0, :] = _repack(a_full[sl])
        xi[:, :, :, 1, :] = _repack(b_full[sl])
        in_maps.append({"x": xi.reshape(TILES, 128, 2 * COLS), "w": wnp})
    nc = _get_program()
    r = run_bass_kernel_spmd(nc, in_maps, list(range(N_CORES)),
                             **spmd_kwargs)
    total = 0.0
    for i in range(N_CORES):
        total += r.results[i]["out"].astype(np.float64).sum()
    return np.float32(total), r


def kernel(original: np.ndarray, pred: np.ndarray) -> np.ndarray:
    val, _ = _run(original, pred)
    return np.array(val, dtype=np.float32)
